# revision 1
# baseline (speedup 1.0000x reference)
"""AbsolutePosEmb attention-logits kernel for 8 Trainium2 NeuronCores.

logits[b,n,x,y,p,q] = sum_d q[b,n,x,y,d] * (k[b,n,p,q,d] + ph[p,d] + pw[q,d])

Strategy: shard the 32 (b,n) pairs across 8 cores (4 pairs/core). Per core,
two pairs are packed into the 128 SBUF partitions (contraction D=64 each, at
base partitions 0/64 -> concurrent PE row-groups). Host supplies q/k already
transposed to [d, hw] fp16; the kernel builds emb^T = ph^T(+)pw^T on-chip,
fuses k' = k + emb on DVE, runs fp16 matmuls (FP22 internal, same mantissa
as fp16 inputs -> products exact, fp32 PSUM accumulate), and streams the
[hw, hw] logit tiles out as fp16 via PSUM->(DVE|ACT)->SBUF->DMA. The host
upcasts to fp32.
"""
import sys
sys.path.insert(0, '/opt/trn_rl_repo')
import numpy as np
import concourse.bass as bass
import concourse.tile as tile
from concourse import bacc, mybir
from concourse import bass_utils

F16 = mybir.dt.float16
F32 = mybir.dt.float32

B, N, H, W, D = 4, 8, 32, 32, 64
HW = H * W
NCORES = 8
PAIRS = (B * N) // NCORES   # 4 (b,n) pairs per core
SP = PAIRS // 2             # 2 super-pairs of 2 partition-packed pairs

CHUNK_PLAN = ((1, 1, 2, 2, 2), (2, 2, 2, 2))  # m-tiles per output DMA
COPY_PATTERN = "VAVAVAVA"                     # PSUM->SBUF copy engine per m
WARM_MM = 6                                   # PE warm-up matmuls


def _build_nc():
    nc = bacc.Bacc("TRN2", target_bir_lowering=False, debug=False,
                   num_devices=NCORES)

    qk = nc.dram_tensor("qk", [SP, 2, 128, HW], F16, kind="ExternalInput")
    phw = nc.dram_tensor("phw", [128, H + W], F32, kind="ExternalInput")
    out = nc.dram_tensor("out", [PAIRS, HW, HW], F16, kind="ExternalOutput")

    max_chunk = max(max(c) for c in CHUNK_PLAN)
    stage_bufs = 2 * (8 // max_chunk)

    with tile.TileContext(nc) as tc:
        with (
            tc.tile_pool(name="cst", bufs=1) as cst,
            tc.tile_pool(name="io", bufs=2) as io,
            tc.tile_pool(name="kp", bufs=2) as kpool,
            tc.tile_pool(name="stage", bufs=stage_bufs) as stage,
            tc.tile_pool(name="ps", bufs=4, space=bass.MemorySpace.PSUM) as ps,
        ):
            # warm-up: PE HAM ramp + ACT activation-table load
            wt = cst.tile([64, 640], F16)
            nc.gpsimd.memset(wt[:], 0.0)
            wact = cst.tile([64, 16], F32)
            nc.gpsimd.memset(wact[:], 0.0)
            wact2 = cst.tile([64, 16], F16)
            nc.scalar.copy(wact2[:], wact[:])

            warm_pt = ps.tile([128, HW], F32, tag="pt", name="warm_pt")
            for _ in range(WARM_MM):
                nc.tensor.matmul(warm_pt[:, 0:512], wt[:, 0:128],
                                 wt[:, 128:640], start=True, stop=True)

            phws = cst.tile([128, H + W], F32)
            nc.sync.dma_start(phws[:], phw.ap())

            # emb^T[d, a*W+b] = ph[a,d] + pw[b,d], rounded to fp16
            emb2 = cst.tile([128, HW], F16)
            nc.vector.tensor_tensor(
                emb2[:].rearrange("p (a b) -> p a b", a=H, b=W),
                phws[:, 0:H].unsqueeze(2).broadcast_to([128, H, W]),
                phws[:, H:H + W].unsqueeze(1).broadcast_to([128, H, W]),
                op=mybir.AluOpType.add,
            )

            for sp in range(SP):
                qkts = io.tile([128, 2 * HW], F16, tag="qkts")
                nc.scalar.dma_start(
                    qkts[:].rearrange("p (t c) -> p t c", t=2),
                    qk[sp].rearrange("t p c -> p t c"),
                )
                qts = qkts[:, 0:HW]
                kps = kpool.tile([128, HW], F16, tag="kps")
                nc.vector.tensor_tensor(kps[:], qkts[:, HW:2 * HW], emb2[:],
                                        op=mybir.AluOpType.add)
                for h in range(2):
                    p = 2 * sp + h
                    chunks = CHUNK_PLAN[0] if p == 0 else CHUNK_PLAN[-1]
                    m = 0
                    for cm in chunks:
                        st = stage.tile([128, cm * HW], F16, tag="st")
                        g0 = m
                        for mi in range(cm):
                            if sp == 0 and h == 0 and m == 0:
                                pt = warm_pt
                            else:
                                pt = ps.tile([128, HW], F32, tag="pt")
                            lhsT = qts[64 * h:64 * (h + 1),
                                       128 * m:128 * (m + 1)]
                            for n in range(2):
                                nc.tensor.matmul(
                                    pt[:, 512 * n:512 * (n + 1)], lhsT,
                                    kps[64 * h:64 * (h + 1),
                                        512 * n:512 * (n + 1)],
                                    start=True, stop=True)
                            dst = st[:, HW * mi:HW * (mi + 1)]
                            if COPY_PATTERN[m % len(COPY_PATTERN)] == "V":
                                nc.vector.tensor_copy(dst, pt[:])
                            else:
                                nc.scalar.copy(dst, pt[:])
                            m += 1
                        nc.sync.dma_start(
                            out[p][128 * g0:128 * (g0 + cm), :]
                                .rearrange("(m x) c -> x m c", m=cm),
                            st[:].rearrange("p (m c) -> p m c", m=cm),
                        )

    nc.compile()
    return nc


_NC_CACHE = []


def kernel(q, k, ph, pw):
    """q,k: [4,8,32,32,64] f32; ph: [32,64] f32; pw: [32,64] f32.
    Returns logits [4,8,32,32,32,32] f32."""
    if not _NC_CACHE:
        _NC_CACHE.append(_build_nc())
    nc = _NC_CACHE[0]

    qt = np.asarray(q, np.float32).astype(np.float16) \
        .reshape(B * N, HW, D).transpose(0, 2, 1)     # [32, 64, 1024]
    kt = np.asarray(k, np.float32).astype(np.float16) \
        .reshape(B * N, HW, D).transpose(0, 2, 1)
    ph = np.asarray(ph, np.float32)
    pw = np.asarray(pw, np.float32)
    phw1 = np.concatenate([ph.T, pw.T], axis=1)       # [64, H+W]
    phw = np.ascontiguousarray(np.vstack([phw1, phw1]), dtype=np.float32)

    in_maps = []
    for c in range(NCORES):
        qc = qt[PAIRS * c:PAIRS * (c + 1)].reshape(SP, 128, HW)
        kc = kt[PAIRS * c:PAIRS * (c + 1)].reshape(SP, 128, HW)
        qkc = np.stack([qc, kc], axis=1)              # [SP, 2, 128, HW]
        in_maps.append({"qk": np.ascontiguousarray(qkc), "phw": phw})

    res = bass_utils.run_bass_kernel_spmd(nc, in_maps,
                                          core_ids=list(range(NCORES)))

    full = np.concatenate(
        [r["out"].astype(np.float32).reshape(PAIRS, H, W, H, W)
         for r in res.results])
    return full.reshape(B, N, H, W, H, W)


# revision 3
# speedup vs baseline: 1.3758x; 1.3758x over previous
"""AbsolutePosEmb attention-logits kernel for 8 Trainium2 NeuronCores.

logits[b,n,x,y,p,q] = sum_d q[b,n,x,y,d] * (k[b,n,p,q,d] + ph[p,d] + pw[q,d])

Strategy: shard the 32 (b,n) pairs across 8 cores (4 pairs/core). Per core,
two pairs are packed into the 128 SBUF partitions (contraction D=64 each, at
base partitions 0/64 -> concurrent PE row-groups). Host supplies q/k already
transposed to [d, hw] fp16; the kernel builds emb^T = ph^T(+)pw^T on-chip,
fuses k' = k + emb on DVE, runs fp16 matmuls (FP22 internal, same mantissa
as fp16 inputs -> products exact, fp32 PSUM accumulate), and streams the
[hw, hw] logit tiles out as fp16 via PSUM->(DVE|ACT)->SBUF->DMA. The host
upcasts to fp32.
"""
import sys
sys.path.insert(0, '/opt/trn_rl_repo')
import numpy as np
import concourse.bass as bass
import concourse.tile as tile
from concourse import bacc, mybir
from concourse import bass_utils

F16 = mybir.dt.float16
F32 = mybir.dt.float32

B, N, H, W, D = 4, 8, 32, 32, 64
HW = H * W
NCORES = 8
PAIRS = (B * N) // NCORES   # 4 (b,n) pairs per core
SP = PAIRS // 2             # 2 super-pairs of 2 partition-packed pairs

CHUNK_PLAN = ((1, 1, 2, 4), (4, 4))  # m-tiles per output DMA
COPY_PATTERN = "VAVAVAVA"                     # PSUM->SBUF copy engine per m
WARM_MM = 6                                   # PE warm-up matmuls


def _build_nc():
    nc = bacc.Bacc("TRN2", target_bir_lowering=False, debug=False,
                   num_devices=NCORES)

    qk = nc.dram_tensor("qk", [SP, 2, 128, HW], F16, kind="ExternalInput")
    phw = nc.dram_tensor("phw", [128, H + W], F32, kind="ExternalInput")
    out = nc.dram_tensor("out", [PAIRS, HW, HW], F16, kind="ExternalOutput")

    max_chunk = max(max(c) for c in CHUNK_PLAN)
    stage_bufs = 2 * (8 // max_chunk)

    with tile.TileContext(nc) as tc:
        with (
            tc.tile_pool(name="cst", bufs=1) as cst,
            tc.tile_pool(name="io", bufs=2) as io,
            tc.tile_pool(name="kp", bufs=2) as kpool,
            tc.tile_pool(name="stage", bufs=stage_bufs) as stage,
            tc.tile_pool(name="ps", bufs=4, space=bass.MemorySpace.PSUM) as ps,
        ):
            # warm-up: PE HAM ramp + ACT activation-table load
            wt = cst.tile([64, 640], F16)
            nc.gpsimd.memset(wt[:], 0.0)
            wact = cst.tile([64, 16], F32)
            nc.gpsimd.memset(wact[:], 0.0)
            wact2 = cst.tile([64, 16], F16)
            nc.scalar.copy(wact2[:], wact[:])

            warm_pt = ps.tile([128, HW], F32, tag="pt", name="warm_pt")
            for _ in range(WARM_MM):
                nc.tensor.matmul(warm_pt[:, 0:512], wt[:, 0:128],
                                 wt[:, 128:640], start=True, stop=True)

            # prefetch all q/k up front (ACT HWDGE ring)
            prefetched = []
            for sp in range(SP):
                qkts = io.tile([128, 2 * HW], F16, tag="qkts",
                               name=f"qkts{sp}")
                nc.scalar.dma_start(
                    qkts[:].rearrange("p (t c) -> p t c", t=2),
                    qk[sp].rearrange("t p c -> p t c"),
                )
                prefetched.append(qkts)

            phws = cst.tile([128, H + W], F32)
            nc.sync.dma_start(phws[:], phw.ap())

            # emb^T[d, a*W+b] = ph[a,d] + pw[b,d], rounded to fp16
            emb2 = cst.tile([128, HW], F16)
            nc.vector.tensor_tensor(
                emb2[:].rearrange("p (a b) -> p a b", a=H, b=W),
                phws[:, 0:H].unsqueeze(2).broadcast_to([128, H, W]),
                phws[:, H:H + W].unsqueeze(1).broadcast_to([128, H, W]),
                op=mybir.AluOpType.add,
            )

            for sp in range(SP):
                qkts = prefetched[sp]
                qts = qkts[:, 0:HW]
                kps = kpool.tile([128, HW], F16, tag="kps")
                nc.vector.tensor_tensor(kps[:], qkts[:, HW:2 * HW], emb2[:],
                                        op=mybir.AluOpType.add)
                for h in range(2):
                    p = 2 * sp + h
                    chunks = CHUNK_PLAN[0] if p == 0 else CHUNK_PLAN[-1]
                    m = 0
                    for cm in chunks:
                        st = stage.tile([128, cm * HW], F16, tag="st")
                        g0 = m
                        for mi in range(cm):
                            if sp == 0 and h == 0 and m == 0:
                                pt = warm_pt
                            else:
                                pt = ps.tile([128, HW], F32, tag="pt")
                            lhsT = qts[64 * h:64 * (h + 1),
                                       128 * m:128 * (m + 1)]
                            for n in range(2):
                                nc.tensor.matmul(
                                    pt[:, 512 * n:512 * (n + 1)], lhsT,
                                    kps[64 * h:64 * (h + 1),
                                        512 * n:512 * (n + 1)],
                                    start=True, stop=True)
                            dst = st[:, HW * mi:HW * (mi + 1)]
                            if COPY_PATTERN[m % len(COPY_PATTERN)] == "V":
                                nc.vector.tensor_copy(dst, pt[:])
                            else:
                                nc.scalar.copy(dst, pt[:])
                            m += 1
                        nc.sync.dma_start(
                            out[p][128 * g0:128 * (g0 + cm), :]
                                .rearrange("(m x) c -> x m c", m=cm),
                            st[:].rearrange("p (m c) -> p m c", m=cm),
                        )

    nc.compile()
    return nc


_NC_CACHE = []


def kernel(q, k, ph, pw):
    """q,k: [4,8,32,32,64] f32; ph: [32,64] f32; pw: [32,64] f32.
    Returns logits [4,8,32,32,32,32] f32."""
    if not _NC_CACHE:
        _NC_CACHE.append(_build_nc())
    nc = _NC_CACHE[0]

    qt = np.asarray(q, np.float32).astype(np.float16) \
        .reshape(B * N, HW, D).transpose(0, 2, 1)     # [32, 64, 1024]
    kt = np.asarray(k, np.float32).astype(np.float16) \
        .reshape(B * N, HW, D).transpose(0, 2, 1)
    ph = np.asarray(ph, np.float32)
    pw = np.asarray(pw, np.float32)
    phw1 = np.concatenate([ph.T, pw.T], axis=1)       # [64, H+W]
    phw = np.ascontiguousarray(np.vstack([phw1, phw1]), dtype=np.float32)

    in_maps = []
    for c in range(NCORES):
        qc = qt[PAIRS * c:PAIRS * (c + 1)].reshape(SP, 128, HW)
        kc = kt[PAIRS * c:PAIRS * (c + 1)].reshape(SP, 128, HW)
        qkc = np.stack([qc, kc], axis=1)              # [SP, 2, 128, HW]
        in_maps.append({"qk": np.ascontiguousarray(qkc), "phw": phw})

    res = bass_utils.run_bass_kernel_spmd(nc, in_maps,
                                          core_ids=list(range(NCORES)))

    full = np.concatenate(
        [r["out"].astype(np.float32).reshape(PAIRS, H, W, H, W)
         for r in res.results])
    return full.reshape(B, N, H, W, H, W)
